# revision 47
# baseline (speedup 1.0000x reference)
# Multi-head causal self-attention with RoPE on 8 NeuronCores (Trainium2).
#
# Sharding: TP-2 x DP-4, zero device communication. Core c handles batch
# b = c//2 and head group g = c%2 (heads 8g..8g+7) over ALL 2048 tokens.
# Each core computes a PARTIAL output projection (its 512 head-dims rows of
# Wo^T); the host sums the two partials per batch during unsharding.
# This removes the K/V-projection duplication of a pure-DP split and makes
# the causal tiling exact (no fully-masked score tiles).
#
# Schedule: attention is Scalar(exp)-paced (~1.05us per 128-kv j-tile) while
# its PE work is only ~0.65us/tile, so all projection / out-projection
# matmul groups live in a BACKLOG that is woven between attention tiles,
# one group per tile, keeping both engines near-saturated:
#   stage0: K(tc0,oc0)+rope, Q(qb0,oc0)+rope, V(tt0-3)     (minimal prefix)
#   qb loop: attention tiles weave [rest of stage0, K/V/Q for qb+1,
#            out-proj(qb-1)]; per-qb batched 1/den (exp(-ln(d))) + selector-
#            matmul broadcast; out-proj feeds the output DMA incrementally.
# Diagonal j-tiles are N-restricted (scores/exp/attnV shrink to the live
# query columns) and masked only on the 128-wide staircase strip, which is
# the same [128,128] triangle for every tile.
#
# Layouts (on chip, bf16 compute / f32 accumulate):
#   q^T, k^T  [128 part = head-pair dims, tokens]    d-major for S^T matmuls
#   vaug      [128 part = tokens, 16 tt, 8 h, 66]    cols = [ones|V(64)|ones]
#   S^T tiles [128 j-tokens, 1024] = h0|h1 halves    softmax along PARTITION
#             via matmul-with-ones; one exp instr covers both heads.
# RoPE uses an "evens-then-odds" permuted head layout (baked into Wq/Wk
# columns host-side) so the rotation partner is a fixed +-32 partition shift.

import sys

import numpy as np
import ml_dtypes

for _p in ("/opt/trn_rl_repo",):
    try:
        import concourse.bass  # noqa: F401
        break
    except ImportError:
        sys.path.insert(0, _p)

import concourse.bass as bass
import concourse.tile as tile
from concourse import mybir
from concourse.bass_utils import run_bass_kernel_spmd

B, T, D, H, DH = 4, 2048, 1024, 16, 64
THETA = 10000.0
NCORES = 8
P = 128
DG = 512   # head dims per core (8 heads)
OC = 4     # 128-wide head-pair chunks per core
DC = 8     # 128-wide input-dim chunks
BLK = 512  # query block width
NQB = 4    # query blocks
NTT = 16   # 128-token tiles

f32 = mybir.dt.float32
bf16 = mybir.dt.bfloat16
BF = ml_dtypes.bfloat16


# ---------------------------------------------------------------- host prep

def _perm():
    """Column permutation: within each head's 64 dims, evens then odds."""
    p = np.empty(D, np.int64)
    for h in range(H):
        for m in range(32):
            p[h * 64 + m] = h * 64 + 2 * m
            p[h * 64 + 32 + m] = h * 64 + 2 * m + 1
    return p


def _rope_tables():
    """cos/sin tables [128, T] for the permuted (evens-first) layout."""
    inv = THETA ** (-(np.arange(0, DH, 2, dtype=np.float64) / DH))  # [32]
    m = np.arange(P) % 64
    fi = m % 32
    pos = np.arange(T)
    ang = pos[None, :].astype(np.float64) * inv[fi][:, None]  # [128, T]
    cos = np.cos(ang)
    sin = np.sin(ang) * np.where(m < 64 // 2, -1.0, 1.0)[:, None]
    return cos.astype(np.float32), sin.astype(np.float32)


def host_prep(x, Wq, bq, Wk, bk, Wv, bv, Wo, bo):
    """Build the 8 per-core input dicts (numpy, bf16)."""
    perm = _perm()
    WqTp = np.ascontiguousarray(Wq.T[:, perm]).astype(BF)
    WkTp = np.ascontiguousarray(Wk.T[:, perm]).astype(BF)
    WvT = np.ascontiguousarray(Wv.T).astype(BF)
    WoT = np.ascontiguousarray(Wo.T).astype(BF)
    bqp = bq[perm].astype(np.float32)
    bkp = bk[perm].astype(np.float32)
    ck, sk = _rope_tables()
    # the staircase mask strip is the same lower triangle for every tile
    r = np.arange(P)
    mk = (r[:, None] <= r[None, :]).astype(np.float32)
    # sel[r, oc, c]: broadcast-selector for the per-(oc) 1/den matmul:
    # row 2oc -> cols 0:64 (head h0), row 2oc+1 -> cols 64:128 (head h1)
    sel = np.zeros((8, OC, P), np.float32)
    for oc in range(OC):
        sel[2 * oc, oc, 0:64] = 1.0
        sel[2 * oc + 1, oc, 64:128] = 1.0
    sel = sel.reshape(8, OC * P)
    in_maps = []
    for c in range(NCORES):
        b, g = c // 2, c % 2
        gs = slice(DG * g, DG * (g + 1))
        in_maps.append({
            "xT": np.ascontiguousarray(x[b].T).astype(BF),
            "WqT": WqTp[:, gs], "WkT": WkTp[:, gs],
            "WvT": np.ascontiguousarray(WvT[:, gs]),
            "WoT": np.ascontiguousarray(WoT[gs, :]),
            "bq": bqp[gs].reshape(1, DG).astype(BF),
            "bk": bkp[gs].reshape(1, DG).astype(BF),
            "bv": bv[gs].reshape(1, DG).astype(BF),
            # host sums the two partials, so each adds half of bo
            "bo": (0.5 * bo).reshape(1, D).astype(BF),
            "ck": ck.astype(BF), "sk": sk.astype(BF),
            "mk": mk.astype(BF), "sel": sel.astype(BF),
        })
    return in_maps


def assemble(results):
    y = np.empty((B, T, D), np.float32)
    for b in range(B):
        y[b] = results[2 * b]["out"] + results[2 * b + 1]["out"]
    return y


# ------------------------------------------------------------- device build

def _legalize_waits(nc, max_waits=1):
    """Limit every instruction to one sync-wait command.

    Walrus's per-instruction structs encode a single sync wait; Tile can
    emit more. For any instruction with k > 1 waits, insert k-1 nops on
    the same engine immediately before it, each carrying one wait —
    position-preserving, so semantics are unchanged.
    """
    eng_obj = {
        mybir.EngineType.PE: nc.tensor,
        mybir.EngineType.Activation: nc.scalar,
        mybir.EngineType.DVE: nc.vector,
        mybir.EngineType.Pool: nc.gpsimd,
        mybir.EngineType.SP: nc.sync,
    }
    fn = nc.m.functions[0]
    for blk in fn.blocks:
        insts = list(blk.instructions)
        new = []
        for inst in insts:
            si = inst.sync_info
            nw = len(si.on_wait) if si is not None else 0
            if nw > max_waits:
                for w in si.on_wait[: nw - max_waits]:
                    eng_obj[inst.engine].nop()
                    nop = fn.blocks[-1].instructions[-1]
                    fn.blocks[-1].instructions = \
                        fn.blocks[-1].instructions[:-1]
                    nop.sync_info = mybir.SyncInfo(on_wait=[w], on_update=[])
                    new.append(nop)
                inst.sync_info = mybir.SyncInfo(
                    on_wait=list(si.on_wait[nw - max_waits:]),
                    on_update=list(si.on_update))
            new.append(inst)
        blk.instructions = new


def build_nc(use_bias):
    from contextlib import ExitStack

    nc = bass.Bass("TRN2", target_bir_lowering=False, debug=False,
                   num_devices=NCORES)
    Exp = mybir.ActivationFunctionType.Exp
    Ln = mybir.ActivationFunctionType.Ln

    xT = nc.dram_tensor("xT", [D, T], bf16, kind="ExternalInput").ap()
    WqT = nc.dram_tensor("WqT", [D, DG], bf16, kind="ExternalInput").ap()
    WkT = nc.dram_tensor("WkT", [D, DG], bf16, kind="ExternalInput").ap()
    WvT = nc.dram_tensor("WvT", [D, DG], bf16, kind="ExternalInput").ap()
    WoT = nc.dram_tensor("WoT", [DG, D], bf16, kind="ExternalInput").ap()
    if use_bias:
        bq_d = nc.dram_tensor("bq", [1, DG], bf16, kind="ExternalInput").ap()
        bk_d = nc.dram_tensor("bk", [1, DG], bf16, kind="ExternalInput").ap()
        bv_d = nc.dram_tensor("bv", [1, DG], bf16, kind="ExternalInput").ap()
        bo_d = nc.dram_tensor("bo", [1, D], bf16, kind="ExternalInput").ap()
    ck_d = nc.dram_tensor("ck", [P, T], bf16, kind="ExternalInput").ap()
    sk_d = nc.dram_tensor("sk", [P, T], bf16, kind="ExternalInput").ap()
    mk_d = nc.dram_tensor("mk", [P, P], bf16, kind="ExternalInput").ap()
    sel_d = nc.dram_tensor("sel", [8, OC * P], bf16,
                           kind="ExternalInput").ap()
    out_d = nc.dram_tensor("out", [T, D], f32, kind="ExternalOutput").ap()

    with tile.TileContext(nc) as tc, ExitStack() as ctx:
        big = ctx.enter_context(tc.tile_pool(name="big", bufs=1))
        const = ctx.enter_context(tc.tile_pool(name="const", bufs=1))
        rpool = ctx.enter_context(tc.tile_pool(name="rp", bufs=2))
        ppool = ctx.enter_context(tc.tile_pool(name="pp", bufs=4))
        npool = ctx.enter_context(tc.tile_pool(name="np", bufs=2))
        outp = ctx.enter_context(tc.tile_pool(name="outp", bufs=3))
        pssc = ctx.enter_context(
            tc.tile_pool(name="pssc", bufs=2, space="PSUM"))
        psacc = ctx.enter_context(
            tc.tile_pool(name="psacc", bufs=1, space="PSUM"))
        psmm = ctx.enter_context(
            tc.tile_pool(name="psmm", bufs=2, space="PSUM"))

        # ---- constants (scalar queue is idle early; demand order)
        ck_s = const.tile([P, T], bf16, tag="ck")
        nc.scalar.dma_start(ck_s, ck_d)
        sk_s = const.tile([P, T], bf16, tag="sk")
        nc.scalar.dma_start(sk_s, sk_d)
        mk_s = const.tile([P, P], bf16, tag="mk")
        nc.scalar.dma_start(mk_s, mk_d)
        if use_bias:
            bq_s = const.tile([1, DG], bf16, tag="bq")
            nc.scalar.dma_start(bq_s, bq_d)
            bk_s = const.tile([1, DG], bf16, tag="bk")
            nc.scalar.dma_start(bk_s, bk_d)
            bv_s = const.tile([1, DG], bf16, tag="bv")
            nc.scalar.dma_start(bv_s, bv_d)
            bo_s = const.tile([1, D], bf16, tag="bo")
            nc.scalar.dma_start(bo_s, bo_d)
            ones512 = const.tile([1, BLK], bf16, tag="ones512")
            nc.vector.memset(ones512, 1.0)
            onesb = const.tile([1, P], bf16, tag="onesb")
            nc.vector.memset(onesb, 1.0)

        # ---- weights / x (demand order: wk+x(tc0) -> wv -> wq -> ...)
        def load_w(src, n_in, n_col, tagp, q):
            tiles = []
            for dc in range(n_in // P):
                t = big.tile([P, n_col], bf16, tag=f"{tagp}{dc}")
                q(t, src[dc * P:(dc + 1) * P, :])
                tiles.append(t)
            return tiles
        # critical first bytes (wk + x tc0) split across two queues so the
        # first K-proj matmul can start as early as possible
        wk_s = []
        for dc in range(DC):
            wt = big.tile([P, DG], bf16, tag=f"wk{dc}")
            q = nc.gpsimd.dma_start if dc % 2 else nc.sync.dma_start
            q(wt, WkT[dc * P:(dc + 1) * P, :])
            wk_s.append(wt)
        x_s = []
        for dc in range(DC):
            xt = big.tile([P, T], bf16, tag=f"x{dc}")
            x_s.append(xt)
        for tcb in range(NQB):
            for dc in range(DC):
                q = nc.gpsimd.dma_start if dc % 2 else nc.sync.dma_start
                q(x_s[dc][:, tcb * BLK:(tcb + 1) * BLK],
                  xT[dc * P:(dc + 1) * P, tcb * BLK:(tcb + 1) * BLK])
        wv_s = load_w(WvT, D, DG, "wv", nc.scalar.dma_start)
        wq_s = load_w(WqT, D, DG, "wq", nc.scalar.dma_start)
        sel_s = const.tile([8, OC, P], bf16, tag="sel")
        nc.scalar.dma_start(sel_s, sel_d.rearrange("r (oc p) -> r oc p", p=P))
        wo_s = load_w(WoT, DG, D, "wo", nc.scalar.dma_start)

        # PE warmup: dummy matmuls with no DMA deps keep the HAM clock
        # gate at full rate until the first real matmul's inputs land
        wrm = const.tile([P, BLK], bf16, tag="wrm")
        nc.vector.memset(wrm, 0.0)
        for _ in range(100):
            wz = psmm.tile([P, BLK], f32, tag="mm")
            nc.tensor.matmul(wz, wrm[:, 0:P], wrm, start=True, stop=True)

        qfin = big.tile([P, OC, T], bf16, tag="qfin")
        kfin = big.tile([P, OC, T], bf16, tag="kfin")
        vaug = big.tile([P, NTT, 8, 66], bf16, tag="vaug")
        nc.vector.memset(vaug[:, :, :, 0:1], 1.0)
        nc.vector.memset(vaug[:, :, :, 65:66], 1.0)
        ctxn = big.tile([P, OC, T], bf16, tag="ctxn")

        def rope(fin, oc, t_lo, wid=BLK):
            # rotate fin[:, oc, t_lo:t_lo+wid] in place (one producer)
            sl = slice(t_lo, t_lo + wid)
            sw = rpool.tile([P, T], bf16, tag="sw")
            for (a, src) in ((0, 32), (32, 0), (64, 96), (96, 64)):
                nc.gpsimd.dma_start(sw[a:a + 32, :wid],
                                    fin[src:src + 32, oc, sl])
            t1 = rpool.tile([P, T], bf16, tag="t1")
            t2 = rpool.tile([P, T], bf16, tag="t2")
            nc.vector.tensor_mul(t1[:, :wid], fin[:, oc, sl], ck_s[:, sl])
            nc.vector.tensor_mul(t2[:, :wid], sw[:, :wid], sk_s[:, sl])
            nc.vector.tensor_add(fin[:, oc, sl], t1[:, :wid], t2[:, :wid])

        def proj_qk(fin, w_tiles, b_s, oc, tcb, rope_now=True):
            # fin[:, oc, tcb*BLK:...] = (W^T x)[dims 128oc.., tokens] + rope
            ps = psmm.tile([P, BLK], f32, tag="mm")
            osl = slice(oc * P, (oc + 1) * P)
            tsl = slice(tcb * BLK, (tcb + 1) * BLK)
            for dc in range(DC):
                nc.tensor.matmul(ps, w_tiles[dc][:, osl], x_s[dc][:, tsl],
                                 start=(dc == 0),
                                 stop=(dc == DC - 1 and not use_bias))
            if use_bias:
                nc.tensor.matmul(ps, b_s[:, osl], ones512,
                                 start=False, stop=True)
            nc.vector.tensor_copy(fin[:, oc, tsl], ps)
            if rope_now:
                rope(fin, oc, tcb * BLK)

        def vproj(tt):
            ps = psmm.tile([P, DG], f32, tag="mm")
            for dc in range(DC):
                nc.tensor.matmul(ps, x_s[dc][:, tt * P:(tt + 1) * P],
                                 wv_s[dc],
                                 start=(dc == 0),
                                 stop=(dc == DC - 1 and not use_bias))
            if use_bias:
                nc.tensor.matmul(ps, onesb, bv_s, start=False, stop=True)
            nc.vector.tensor_copy(vaug[:, tt, 0:8, 1:65], ps)

        def oproj(qb, tp, half):
            tsl = slice(qb * BLK + tp * P, qb * BLK + (tp + 1) * P)
            esl = slice(half * BLK, (half + 1) * BLK)
            ps = psmm.tile([P, BLK], f32, tag="mm")
            for oc in range(OC):
                nc.tensor.matmul(ps, ctxn[:, oc, tsl], wo_s[oc][:, esl],
                                 start=(oc == 0),
                                 stop=(oc == OC - 1 and not use_bias))
            if use_bias:
                nc.tensor.matmul(ps, onesb, bo_s[:, esl],
                                 start=False, stop=True)
            ot = outp.tile([P, BLK], f32, tag="ot")
            nc.vector.tensor_copy(ot, ps)
            nc.sync.dma_start(out_d[tsl, esl], ot)

        def strip(ap2d, co):
            # cols [co:co+128] and [BLK+co:BLK+co+128] of a [128, 2*BLK] AP
            s = ap2d[:, co:]
            return bass.AP(tensor=s.tensor, offset=s.offset,
                           ap=[s.ap[0], [BLK, 2], [1, P]])

        def tri_mask():
            # the [128,128] triangle, read twice via a 0-stride middle dim
            s = mk_s[:, :]
            return bass.AP(tensor=s.tensor, offset=s.offset,
                           ap=[s.ap[0], [0, 2], [1, P]])

        # ---- stage: K proj (token-block outer, demand-ordered vs the x
        # DMA stream), batched full-T RoPE, V proj, Q proj(qb0)
        for tcb in range(NQB):
            for oc in range(OC):
                proj_qk(kfin, wk_s, bk_s if use_bias else None, oc, tcb,
                        rope_now=False)
        for oc in range(OC):
            rope(kfin, oc, 0, T)
        for tt in range(NTT):
            vproj(tt)
        for oc in range(OC):
            proj_qk(qfin, wq_s, bq_s if use_bias else None, oc, 0)

        # out-proj groups of the previous qb drip into this qb's attention
        drip = []

        # ---- main pipeline over query blocks
        for qb in range(NQB):
            qsl = slice(qb * BLK, (qb + 1) * BLK)
            J = 4 * qb + 4
            ntiles = OC * J
            period = max(1, ntiles // (len(drip) + 1))
            tile_i = 0
            stg = npool.tile([8, BLK], f32, tag="stg")
            for oc in range(OC):
                opsA = psacc.tile([P, BLK], f32, tag="opsA")
                opsB = psacc.tile([P, BLK], f32, tag="opsB")
                for jt in range(J):
                    # diagonal tiles: queries [0, co) of this block can't
                    # see kv tile jt — shrink every op to cols [co, BLK)
                    jl = jt - 4 * qb
                    co = jl * P if jl > 0 else 0
                    qco = slice(qb * BLK + co, (qb + 1) * BLK)
                    sAB = pssc.tile([P, 2 * BLK], f32, tag="sAB")
                    nc.tensor.matmul(
                        sAB[:, co:BLK],
                        kfin[0:64, oc, jt * P:(jt + 1) * P],
                        qfin[0:64, oc, qco],
                        start=True, stop=True, tile_position=(0, 0))
                    nc.tensor.matmul(
                        sAB[:, BLK + co:2 * BLK],
                        kfin[64:128, oc, jt * P:(jt + 1) * P],
                        qfin[64:128, oc, qco],
                        start=True, stop=True, tile_position=(64, 0))
                    pAB = ppool.tile([P, 2 * BLK], bf16, tag="pAB")
                    if co > 0:
                        s_in = sAB[:, co:]
                        s_src = bass.AP(
                            tensor=s_in.tensor, offset=s_in.offset,
                            ap=[s_in.ap[0], [BLK, 2], [1, BLK - co]])
                        p_out = pAB[:, co:]
                        p_dst = bass.AP(
                            tensor=p_out.tensor, offset=p_out.offset,
                            ap=[p_out.ap[0], [BLK, 2], [1, BLK - co]])
                        nc.scalar.activation(p_dst, s_src, Exp, scale=0.125)
                    else:
                        nc.scalar.activation(pAB, sAB, Exp, scale=0.125)
                    if jl >= 0:
                        # staircase mask only on the diagonal strip
                        nc.vector.tensor_mul(strip(pAB, co), strip(pAB, co),
                                             tri_mask())
                    nc.tensor.matmul(opsA[0:65, co:BLK],
                                     vaug[:, jt, 2 * oc, 1:66],
                                     pAB[:, co:BLK],
                                     start=(jt == 0), stop=(jt == J - 1))
                    nc.tensor.matmul(opsB[0:65, co:BLK],
                                     vaug[:, jt, 2 * oc + 1, 1:66],
                                     pAB[:, BLK + co:2 * BLK],
                                     start=(jt == 0), stop=(jt == J - 1))
                    tile_i += 1
                    if len(drip) > 2 and tile_i % period == 0:
                        drip.pop(0)()
                # evacuate unnormalized ctx (releases opsA/B for next oc)
                # and stage the two denominator rows into rows 2oc,2oc+1
                nc.vector.tensor_copy(ctxn[0:64, oc, qsl], opsA[0:64, :])
                nc.vector.tensor_copy(ctxn[64:128, oc, qsl], opsB[0:64, :])
                for (hh, ops) in ((0, opsA), (1, opsB)):
                    dt = npool.tile([1, BLK], f32, tag="dtmp")
                    nc.vector.tensor_copy(dt, ops[64:65, :])
                    nc.sync.dma_start(stg[2 * oc + hh:2 * oc + hh + 1, :], dt)
            for g in drip:
                g()
            drip = []
            # Q proj for the next block covers the normalize chain
            if qb < NQB - 1:
                for oc in range(OC):
                    proj_qk(qfin, wq_s, bq_s if use_bias else None,
                            oc, qb + 1)
            # batched reciprocal: rec = exp(-ln(den)), then per-oc broadcast
            lnd = npool.tile([8, BLK], f32, tag="lnd")
            nc.scalar.activation(lnd, stg, Ln)
            rec = npool.tile([8, BLK], bf16, tag="rec")
            nc.scalar.activation(rec, lnd, Exp, scale=-1.0)
            for oc in range(OC):
                bc = psmm.tile([P, BLK], f32, tag="mm")
                nc.tensor.matmul(bc, sel_s[:, oc, :], rec,
                                 start=True, stop=True)
                nc.vector.tensor_mul(ctxn[0:64, oc, qsl],
                                     ctxn[0:64, oc, qsl], bc[0:64, :])
                nc.vector.tensor_mul(ctxn[64:128, oc, qsl],
                                     ctxn[64:128, oc, qsl], bc[64:128, :])
            # out-proj groups drip into the next qb's attention
            for tp in range(4):
                for half in range(2):
                    drip.append(lambda q=qb, tp=tp, h=half: oproj(q, tp, h))
        for g in drip:
            g()
    _legalize_waits(nc)
    return nc


# ------------------------------------------------------------------- entry

def kernel(x, Wq, bq, Wk, bk, Wv, bv, Wo, bo):
    x = np.asarray(x, np.float32)
    Wq, bq = np.asarray(Wq, np.float32), np.asarray(bq, np.float32)
    Wk, bk = np.asarray(Wk, np.float32), np.asarray(bk, np.float32)
    Wv, bv = np.asarray(Wv, np.float32), np.asarray(bv, np.float32)
    Wo, bo = np.asarray(Wo, np.float32), np.asarray(bo, np.float32)
    use_bias = bool(any(np.any(b) for b in (bq, bk, bv, bo)))
    in_maps = host_prep(x, Wq, bq, Wk, bk, Wv, bv, Wo, bo)
    if not use_bias:
        for m in in_maps:
            for k in ("bq", "bk", "bv", "bo"):
                m.pop(k)
    nc = build_nc(use_bias)
    res = run_bass_kernel_spmd(nc, in_maps, list(range(NCORES))).results
    return assemble(res)


# revision 50
# speedup vs baseline: 1.1895x; 1.1895x over previous
# Multi-head causal self-attention with RoPE on 8 NeuronCores (Trainium2).
#
# Sharding: TP-2 x DP-4, zero device communication. Core c handles batch
# b = c//2 and head group g = c%2 (heads 8g..8g+7) over ALL 2048 tokens.
# Each core computes a PARTIAL output projection (its 512 head-dims rows of
# Wo^T); the host sums the two partials per batch during unsharding.
# This removes the K/V-projection duplication of a pure-DP split and makes
# the causal tiling exact (no fully-masked score tiles).
#
# Schedule: attention is Scalar(exp)-paced (~1.05us per 128-kv j-tile) while
# its PE work is only ~0.65us/tile, so all projection / out-projection
# matmul groups live in a BACKLOG that is woven between attention tiles,
# one group per tile, keeping both engines near-saturated:
#   stage0: K(tc0,oc0)+rope, Q(qb0,oc0)+rope, V(tt0-3)     (minimal prefix)
#   qb loop: attention tiles weave [rest of stage0, K/V/Q for qb+1,
#            out-proj(qb-1)]; per-qb batched 1/den (exp(-ln(d))) + selector-
#            matmul broadcast; out-proj feeds the output DMA incrementally.
# Diagonal j-tiles are N-restricted (scores/exp/attnV shrink to the live
# query columns) and masked only on the 128-wide staircase strip, which is
# the same [128,128] triangle for every tile.
#
# Layouts (on chip, bf16 compute / f32 accumulate):
#   q^T, k^T  [128 part = head-pair dims, tokens]    d-major for S^T matmuls
#   vaug      [128 part = tokens, 16 tt, 8 h, 66]    cols = [ones|V(64)|ones]
#   S^T tiles [128 j-tokens, 1024] = h0|h1 halves    softmax along PARTITION
#             via matmul-with-ones; one exp instr covers both heads.
# RoPE uses an "evens-then-odds" permuted head layout (baked into Wq/Wk
# columns host-side) so the rotation partner is a fixed +-32 partition shift.

import sys

import numpy as np
import ml_dtypes

for _p in ("/opt/trn_rl_repo",):
    try:
        import concourse.bass  # noqa: F401
        break
    except ImportError:
        sys.path.insert(0, _p)

import concourse.bass as bass
import concourse.tile as tile
from concourse import mybir
from concourse.bass_utils import run_bass_kernel_spmd

B, T, D, H, DH = 4, 2048, 1024, 16, 64
THETA = 10000.0
NCORES = 8
P = 128
DG = 512   # head dims per core (8 heads)
OC = 4     # 128-wide head-pair chunks per core
DC = 8     # 128-wide input-dim chunks
BLK = 512  # query block width
NQB = 4    # query blocks
NTT = 16   # 128-token tiles

f32 = mybir.dt.float32
bf16 = mybir.dt.bfloat16
BF = ml_dtypes.bfloat16


# ---------------------------------------------------------------- host prep

def _perm():
    """Column permutation: within each head's 64 dims, evens then odds."""
    p = np.empty(D, np.int64)
    for h in range(H):
        for m in range(32):
            p[h * 64 + m] = h * 64 + 2 * m
            p[h * 64 + 32 + m] = h * 64 + 2 * m + 1
    return p


def _rope_tables():
    """cos/sin tables [128, T] for the permuted (evens-first) layout."""
    inv = THETA ** (-(np.arange(0, DH, 2, dtype=np.float64) / DH))  # [32]
    m = np.arange(P) % 64
    fi = m % 32
    pos = np.arange(T)
    ang = pos[None, :].astype(np.float64) * inv[fi][:, None]  # [128, T]
    cos = np.cos(ang)
    sin = np.sin(ang) * np.where(m < 64 // 2, -1.0, 1.0)[:, None]
    return cos.astype(np.float32), sin.astype(np.float32)


def host_prep(x, Wq, bq, Wk, bk, Wv, bv, Wo, bo):
    """Build the 8 per-core input dicts (numpy, bf16)."""
    perm = _perm()
    WqTp = np.ascontiguousarray(Wq.T[:, perm]).astype(BF)
    WkTp = np.ascontiguousarray(Wk.T[:, perm]).astype(BF)
    WvT = np.ascontiguousarray(Wv.T).astype(BF)
    WoT = np.ascontiguousarray(Wo.T).astype(BF)
    bqp = bq[perm].astype(np.float32)
    bkp = bk[perm].astype(np.float32)
    ck, sk = _rope_tables()
    # the staircase mask strip is the same lower triangle for every tile
    r = np.arange(P)
    mk = (r[:, None] <= r[None, :]).astype(np.float32)
    # sel[r, oc, c]: broadcast-selector for the per-(oc) 1/den matmul:
    # row 2oc -> cols 0:64 (head h0), row 2oc+1 -> cols 64:128 (head h1)
    sel = np.zeros((8, OC, P), np.float32)
    for oc in range(OC):
        sel[2 * oc, oc, 0:64] = 1.0
        sel[2 * oc + 1, oc, 64:128] = 1.0
    sel = sel.reshape(8, OC * P)
    in_maps = []
    for c in range(NCORES):
        b, g = c // 2, c % 2
        gs = slice(DG * g, DG * (g + 1))
        in_maps.append({
            "xT": np.ascontiguousarray(x[b].T).astype(BF),
            "WqT": WqTp[:, gs], "WkT": WkTp[:, gs],
            "WvT": np.ascontiguousarray(WvT[:, gs]),
            "WoT": np.ascontiguousarray(WoT[gs, :]),
            "bq": bqp[gs].reshape(1, DG).astype(BF),
            "bk": bkp[gs].reshape(1, DG).astype(BF),
            "bv": bv[gs].reshape(1, DG).astype(BF),
            # host sums the two partials, so each adds half of bo
            "bo": (0.5 * bo).reshape(1, D).astype(BF),
            "ck": ck.astype(BF), "sk": sk.astype(BF),
            "mk": mk.astype(BF), "sel": sel.astype(BF),
        })
    return in_maps


def assemble(results):
    y = np.empty((B, T, D), np.float32)
    for b in range(B):
        y[b] = results[2 * b]["out"] + results[2 * b + 1]["out"]
    return y


# ------------------------------------------------------------- device build

def _legalize_waits(nc, max_waits=1):
    """Limit every instruction to one sync-wait command.

    Walrus's per-instruction structs encode a single sync wait; Tile can
    emit more. For any instruction with k > 1 waits, insert k-1 nops on
    the same engine immediately before it, each carrying one wait —
    position-preserving, so semantics are unchanged.
    """
    eng_obj = {
        mybir.EngineType.PE: nc.tensor,
        mybir.EngineType.Activation: nc.scalar,
        mybir.EngineType.DVE: nc.vector,
        mybir.EngineType.Pool: nc.gpsimd,
        mybir.EngineType.SP: nc.sync,
    }
    fn = nc.m.functions[0]
    for blk in fn.blocks:
        insts = list(blk.instructions)
        new = []
        for inst in insts:
            si = inst.sync_info
            nw = len(si.on_wait) if si is not None else 0
            if nw > max_waits:
                for w in si.on_wait[: nw - max_waits]:
                    eng_obj[inst.engine].nop()
                    nop = fn.blocks[-1].instructions[-1]
                    fn.blocks[-1].instructions = \
                        fn.blocks[-1].instructions[:-1]
                    nop.sync_info = mybir.SyncInfo(on_wait=[w], on_update=[])
                    new.append(nop)
                inst.sync_info = mybir.SyncInfo(
                    on_wait=list(si.on_wait[nw - max_waits:]),
                    on_update=list(si.on_update))
            new.append(inst)
        blk.instructions = new


def build_nc(use_bias):
    from contextlib import ExitStack

    nc = bass.Bass("TRN2", target_bir_lowering=False, debug=False,
                   num_devices=NCORES)
    Exp = mybir.ActivationFunctionType.Exp
    Ln = mybir.ActivationFunctionType.Ln

    xT = nc.dram_tensor("xT", [D, T], bf16, kind="ExternalInput").ap()
    WqT = nc.dram_tensor("WqT", [D, DG], bf16, kind="ExternalInput").ap()
    WkT = nc.dram_tensor("WkT", [D, DG], bf16, kind="ExternalInput").ap()
    WvT = nc.dram_tensor("WvT", [D, DG], bf16, kind="ExternalInput").ap()
    WoT = nc.dram_tensor("WoT", [DG, D], bf16, kind="ExternalInput").ap()
    if use_bias:
        bq_d = nc.dram_tensor("bq", [1, DG], bf16, kind="ExternalInput").ap()
        bk_d = nc.dram_tensor("bk", [1, DG], bf16, kind="ExternalInput").ap()
        bv_d = nc.dram_tensor("bv", [1, DG], bf16, kind="ExternalInput").ap()
        bo_d = nc.dram_tensor("bo", [1, D], bf16, kind="ExternalInput").ap()
    ck_d = nc.dram_tensor("ck", [P, T], bf16, kind="ExternalInput").ap()
    sk_d = nc.dram_tensor("sk", [P, T], bf16, kind="ExternalInput").ap()
    mk_d = nc.dram_tensor("mk", [P, P], bf16, kind="ExternalInput").ap()
    sel_d = nc.dram_tensor("sel", [8, OC * P], bf16,
                           kind="ExternalInput").ap()
    out_d = nc.dram_tensor("out", [T, D], f32, kind="ExternalOutput").ap()

    with tile.TileContext(nc) as tc, ExitStack() as ctx:
        big = ctx.enter_context(tc.tile_pool(name="big", bufs=1))
        const = ctx.enter_context(tc.tile_pool(name="const", bufs=1))
        rpool = ctx.enter_context(tc.tile_pool(name="rp", bufs=2))
        ppool = ctx.enter_context(tc.tile_pool(name="pp", bufs=4))
        npool = ctx.enter_context(tc.tile_pool(name="np", bufs=2))
        outp = ctx.enter_context(tc.tile_pool(name="outp", bufs=3))
        pssc = ctx.enter_context(
            tc.tile_pool(name="pssc", bufs=2, space="PSUM"))
        psacc = ctx.enter_context(
            tc.tile_pool(name="psacc", bufs=1, space="PSUM"))
        psmm = ctx.enter_context(
            tc.tile_pool(name="psmm", bufs=2, space="PSUM"))

        # ---- constants (scalar queue is idle early; demand order)
        ck_s = const.tile([P, T], bf16, tag="ck")
        nc.scalar.dma_start(ck_s, ck_d)
        sk_s = const.tile([P, T], bf16, tag="sk")
        nc.scalar.dma_start(sk_s, sk_d)
        mk_s = const.tile([P, P], bf16, tag="mk")
        nc.scalar.dma_start(mk_s, mk_d)
        if use_bias:
            bq_s = const.tile([1, DG], bf16, tag="bq")
            nc.scalar.dma_start(bq_s, bq_d)
            bk_s = const.tile([1, DG], bf16, tag="bk")
            nc.scalar.dma_start(bk_s, bk_d)
            bv_s = const.tile([1, DG], bf16, tag="bv")
            nc.scalar.dma_start(bv_s, bv_d)
            bo_s = const.tile([1, D], bf16, tag="bo")
            nc.scalar.dma_start(bo_s, bo_d)
            ones512 = const.tile([1, BLK], bf16, tag="ones512")
            nc.vector.memset(ones512, 1.0)
            onesb = const.tile([1, P], bf16, tag="onesb")
            nc.vector.memset(onesb, 1.0)

        # ---- weights / x (demand order: wk+x(tc0) -> wv -> wq -> ...)
        def load_w(src, n_in, n_col, tagp, q):
            tiles = []
            for dc in range(n_in // P):
                t = big.tile([P, n_col], bf16, tag=f"{tagp}{dc}")
                q(t, src[dc * P:(dc + 1) * P, :])
                tiles.append(t)
            return tiles
        # critical first bytes (wk oc0-columns + x tc0) split across two
        # queues so the first K-proj matmul can start as early as possible
        wk_s = []
        for dc in range(DC):
            wt = big.tile([P, DG], bf16, tag=f"wk{dc}")
            wk_s.append(wt)
        for oc in range(OC):
            for dc in range(DC):
                q = nc.gpsimd.dma_start if dc % 2 else nc.sync.dma_start
                q(wk_s[dc][:, oc * P:(oc + 1) * P],
                  WkT[dc * P:(dc + 1) * P, oc * P:(oc + 1) * P])
        x_s = []
        for dc in range(DC):
            xt = big.tile([P, T], bf16, tag=f"x{dc}")
            x_s.append(xt)
        for tcb in range(NQB):
            for dc in range(DC):
                q = nc.gpsimd.dma_start if dc % 2 else nc.sync.dma_start
                q(x_s[dc][:, tcb * BLK:(tcb + 1) * BLK],
                  xT[dc * P:(dc + 1) * P, tcb * BLK:(tcb + 1) * BLK])
        wv_s = load_w(WvT, D, DG, "wv", nc.scalar.dma_start)
        wq_s = load_w(WqT, D, DG, "wq", nc.scalar.dma_start)
        sel_s = const.tile([8, OC, P], bf16, tag="sel")
        nc.scalar.dma_start(sel_s, sel_d.rearrange("r (oc p) -> r oc p", p=P))
        wo_s = load_w(WoT, DG, D, "wo", nc.scalar.dma_start)

        qfin = big.tile([P, OC, T], bf16, tag="qfin")
        kfin = big.tile([P, OC, T], bf16, tag="kfin")
        vaug = big.tile([P, NTT, 8, 66], bf16, tag="vaug")
        nc.vector.memset(vaug[:, :, :, 0:1], 1.0)
        nc.vector.memset(vaug[:, :, :, 65:66], 1.0)
        ctxn = big.tile([P, OC, T], bf16, tag="ctxn")

        def rope(fin, oc, t_lo, wid=BLK):
            # rotate fin[:, oc, t_lo:t_lo+wid] in place (one producer)
            sl = slice(t_lo, t_lo + wid)
            sw = rpool.tile([P, T], bf16, tag="sw")
            for (a, src) in ((0, 32), (32, 0), (64, 96), (96, 64)):
                nc.gpsimd.dma_start(sw[a:a + 32, :wid],
                                    fin[src:src + 32, oc, sl])
            t1 = rpool.tile([P, T], bf16, tag="t1")
            t2 = rpool.tile([P, T], bf16, tag="t2")
            nc.vector.tensor_mul(t1[:, :wid], fin[:, oc, sl], ck_s[:, sl])
            nc.vector.tensor_mul(t2[:, :wid], sw[:, :wid], sk_s[:, sl])
            nc.vector.tensor_add(fin[:, oc, sl], t1[:, :wid], t2[:, :wid])

        def proj_qk(fin, w_tiles, b_s, oc, tcb, rope_now=True):
            # fin[:, oc, tcb*BLK:...] = (W^T x)[dims 128oc.., tokens] + rope
            ps = psmm.tile([P, BLK], f32, tag="mm")
            osl = slice(oc * P, (oc + 1) * P)
            tsl = slice(tcb * BLK, (tcb + 1) * BLK)
            for dc in range(DC):
                nc.tensor.matmul(ps, w_tiles[dc][:, osl], x_s[dc][:, tsl],
                                 start=(dc == 0),
                                 stop=(dc == DC - 1 and not use_bias))
            if use_bias:
                nc.tensor.matmul(ps, b_s[:, osl], ones512,
                                 start=False, stop=True)
            nc.vector.tensor_copy(fin[:, oc, tsl], ps)
            if rope_now:
                rope(fin, oc, tcb * BLK)

        def vproj(tt):
            ps = psmm.tile([P, DG], f32, tag="mm")
            for dc in range(DC):
                nc.tensor.matmul(ps, x_s[dc][:, tt * P:(tt + 1) * P],
                                 wv_s[dc],
                                 start=(dc == 0),
                                 stop=(dc == DC - 1 and not use_bias))
            if use_bias:
                nc.tensor.matmul(ps, onesb, bv_s, start=False, stop=True)
            nc.vector.tensor_copy(vaug[:, tt, 0:8, 1:65], ps)

        def oproj(qb, tp, half):
            tsl = slice(qb * BLK + tp * P, qb * BLK + (tp + 1) * P)
            esl = slice(half * BLK, (half + 1) * BLK)
            ps = psmm.tile([P, BLK], f32, tag="mm")
            for oc in range(OC):
                nc.tensor.matmul(ps, ctxn[:, oc, tsl], wo_s[oc][:, esl],
                                 start=(oc == 0),
                                 stop=(oc == OC - 1 and not use_bias))
            if use_bias:
                nc.tensor.matmul(ps, onesb, bo_s[:, esl],
                                 start=False, stop=True)
            ot = outp.tile([P, BLK], f32, tag="ot")
            nc.vector.tensor_copy(ot, ps)
            nc.sync.dma_start(out_d[tsl, esl], ot)

        def strip(ap2d, co):
            # cols [co:co+128] and [BLK+co:BLK+co+128] of a [128, 2*BLK] AP
            s = ap2d[:, co:]
            return bass.AP(tensor=s.tensor, offset=s.offset,
                           ap=[s.ap[0], [BLK, 2], [1, P]])

        def tri_mask():
            # the [128,128] triangle, read twice via a 0-stride middle dim
            s = mk_s[:, :]
            return bass.AP(tensor=s.tensor, offset=s.offset,
                           ap=[s.ap[0], [0, 2], [1, P]])

        # ---- stage: K proj (token-block outer, demand-ordered vs the x
        # DMA stream), batched full-T RoPE, V proj, Q proj(qb0)
        for tcb in range(NQB):
            for oc in range(OC):
                proj_qk(kfin, wk_s, bk_s if use_bias else None, oc, tcb,
                        rope_now=False)
        for oc in range(OC):
            rope(kfin, oc, 0, T)
        for tt in range(8):
            vproj(tt)
        for oc in range(OC):
            proj_qk(qfin, wq_s, bq_s if use_bias else None, oc, 0)

        # drip: PE work emitted between attention tiles (scalar paces
        # there). V for tokens [1024,2048) isn't needed until qb2.
        drip = [(lambda tt=tt: vproj(tt)) for tt in range(8, NTT)]

        # ---- main pipeline over query blocks
        for qb in range(NQB):
            qsl = slice(qb * BLK, (qb + 1) * BLK)
            J = 4 * qb + 4
            ntiles = OC * J
            period = max(1, ntiles // (len(drip) + 1))
            tile_i = 0
            stg = npool.tile([8, BLK], f32, tag="stg")
            for oc in range(OC):
                opsA = psacc.tile([P, BLK], f32, tag="opsA")
                opsB = psacc.tile([P, BLK], f32, tag="opsB")
                for jt in range(J):
                    # diagonal tiles: queries [0, co) of this block can't
                    # see kv tile jt — shrink every op to cols [co, BLK)
                    jl = jt - 4 * qb
                    co = jl * P if jl > 0 else 0
                    qco = slice(qb * BLK + co, (qb + 1) * BLK)
                    sAB = pssc.tile([P, 2 * BLK], f32, tag="sAB")
                    nc.tensor.matmul(
                        sAB[:, co:BLK],
                        kfin[0:64, oc, jt * P:(jt + 1) * P],
                        qfin[0:64, oc, qco],
                        start=True, stop=True, tile_position=(0, 0))
                    nc.tensor.matmul(
                        sAB[:, BLK + co:2 * BLK],
                        kfin[64:128, oc, jt * P:(jt + 1) * P],
                        qfin[64:128, oc, qco],
                        start=True, stop=True, tile_position=(64, 0))
                    pAB = ppool.tile([P, 2 * BLK], bf16, tag="pAB")
                    if co > 0:
                        s_in = sAB[:, co:]
                        s_src = bass.AP(
                            tensor=s_in.tensor, offset=s_in.offset,
                            ap=[s_in.ap[0], [BLK, 2], [1, BLK - co]])
                        p_out = pAB[:, co:]
                        p_dst = bass.AP(
                            tensor=p_out.tensor, offset=p_out.offset,
                            ap=[p_out.ap[0], [BLK, 2], [1, BLK - co]])
                        nc.scalar.activation(p_dst, s_src, Exp, scale=0.125)
                    else:
                        nc.scalar.activation(pAB, sAB, Exp, scale=0.125)
                    if jl >= 0:
                        # staircase mask only on the diagonal strip
                        nc.vector.tensor_mul(strip(pAB, co), strip(pAB, co),
                                             tri_mask())
                    nc.tensor.matmul(opsA[0:65, co:BLK],
                                     vaug[:, jt, 2 * oc, 1:66],
                                     pAB[:, co:BLK],
                                     start=(jt == 0), stop=(jt == J - 1))
                    nc.tensor.matmul(opsB[0:65, co:BLK],
                                     vaug[:, jt, 2 * oc + 1, 1:66],
                                     pAB[:, BLK + co:2 * BLK],
                                     start=(jt == 0), stop=(jt == J - 1))
                    tile_i += 1
                    if len(drip) > 2 and tile_i % period == 0:
                        drip.pop(0)()
                # evacuate unnormalized ctx (releases opsA/B for next oc)
                # and stage the two denominator rows into rows 2oc,2oc+1
                nc.vector.tensor_copy(ctxn[0:64, oc, qsl], opsA[0:64, :])
                nc.vector.tensor_copy(ctxn[64:128, oc, qsl], opsB[0:64, :])
                for (hh, ops) in ((0, opsA), (1, opsB)):
                    dt = npool.tile([1, BLK], f32, tag="dtmp")
                    nc.vector.tensor_copy(dt, ops[64:65, :])
                    nc.sync.dma_start(stg[2 * oc + hh:2 * oc + hh + 1, :], dt)
            for g in drip:
                g()
            drip = []
            # Q proj for the next block covers the normalize chain
            if qb < NQB - 1:
                for oc in range(OC):
                    proj_qk(qfin, wq_s, bq_s if use_bias else None,
                            oc, qb + 1)
            # batched reciprocal: rec = exp(-ln(den)), then per-oc broadcast
            lnd = npool.tile([8, BLK], f32, tag="lnd")
            nc.scalar.activation(lnd, stg, Ln)
            rec = npool.tile([8, BLK], bf16, tag="rec")
            nc.scalar.activation(rec, lnd, Exp, scale=-1.0)
            for oc in range(OC):
                bc = psmm.tile([P, BLK], f32, tag="mm")
                nc.tensor.matmul(bc, sel_s[:, oc, :], rec,
                                 start=True, stop=True)
                nc.vector.tensor_mul(ctxn[0:64, oc, qsl],
                                     ctxn[0:64, oc, qsl], bc[0:64, :])
                nc.vector.tensor_mul(ctxn[64:128, oc, qsl],
                                     ctxn[64:128, oc, qsl], bc[64:128, :])
            # out-proj groups drip into the next qb's attention
            for tp in range(4):
                for half in range(2):
                    drip.append(lambda q=qb, tp=tp, h=half: oproj(q, tp, h))
        for g in drip:
            g()
    _legalize_waits(nc)
    return nc


# ------------------------------------------------------------------- entry

def kernel(x, Wq, bq, Wk, bk, Wv, bv, Wo, bo):
    x = np.asarray(x, np.float32)
    Wq, bq = np.asarray(Wq, np.float32), np.asarray(bq, np.float32)
    Wk, bk = np.asarray(Wk, np.float32), np.asarray(bk, np.float32)
    Wv, bv = np.asarray(Wv, np.float32), np.asarray(bv, np.float32)
    Wo, bo = np.asarray(Wo, np.float32), np.asarray(bo, np.float32)
    use_bias = bool(any(np.any(b) for b in (bq, bk, bv, bo)))
    in_maps = host_prep(x, Wq, bq, Wk, bk, Wv, bv, Wo, bo)
    if not use_bias:
        for m in in_maps:
            for k in ("bq", "bk", "bv", "bo"):
                m.pop(k)
    nc = build_nc(use_bias)
    res = run_bass_kernel_spmd(nc, in_maps, list(range(NCORES))).results
    return assemble(res)


# revision 51
# speedup vs baseline: 1.2264x; 1.0310x over previous
# Multi-head causal self-attention with RoPE on 8 NeuronCores (Trainium2).
#
# Sharding: TP-2 x DP-4, zero device communication. Core c handles batch
# b = c//2 and head group g = c%2 (heads 8g..8g+7) over ALL 2048 tokens.
# Each core computes a PARTIAL output projection (its 512 head-dims rows of
# Wo^T); the host sums the two partials per batch during unsharding.
# This removes the K/V-projection duplication of a pure-DP split and makes
# the causal tiling exact (no fully-masked score tiles).
#
# Schedule: attention is Scalar(exp)-paced (~1.05us per 128-kv j-tile) while
# its PE work is only ~0.65us/tile, so all projection / out-projection
# matmul groups live in a BACKLOG that is woven between attention tiles,
# one group per tile, keeping both engines near-saturated:
#   stage0: K(tc0,oc0)+rope, Q(qb0,oc0)+rope, V(tt0-3)     (minimal prefix)
#   qb loop: attention tiles weave [rest of stage0, K/V/Q for qb+1,
#            out-proj(qb-1)]; per-qb batched 1/den (exp(-ln(d))) + selector-
#            matmul broadcast; out-proj feeds the output DMA incrementally.
# Diagonal j-tiles are N-restricted (scores/exp/attnV shrink to the live
# query columns) and masked only on the 128-wide staircase strip, which is
# the same [128,128] triangle for every tile.
#
# Layouts (on chip, bf16 compute / f32 accumulate):
#   q^T, k^T  [128 part = head-pair dims, tokens]    d-major for S^T matmuls
#   vaug      [128 part = tokens, 16 tt, 8 h, 66]    cols = [ones|V(64)|ones]
#   S^T tiles [128 j-tokens, 1024] = h0|h1 halves    softmax along PARTITION
#             via matmul-with-ones; one exp instr covers both heads.
# RoPE uses an "evens-then-odds" permuted head layout (baked into Wq/Wk
# columns host-side) so the rotation partner is a fixed +-32 partition shift.

import sys

import numpy as np
import ml_dtypes

for _p in ("/opt/trn_rl_repo",):
    try:
        import concourse.bass  # noqa: F401
        break
    except ImportError:
        sys.path.insert(0, _p)

import concourse.bass as bass
import concourse.tile as tile
from concourse import mybir
from concourse.bass_utils import run_bass_kernel_spmd

B, T, D, H, DH = 4, 2048, 1024, 16, 64
THETA = 10000.0
NCORES = 8
P = 128
DG = 512   # head dims per core (8 heads)
OC = 4     # 128-wide head-pair chunks per core
DC = 8     # 128-wide input-dim chunks
BLK = 512  # query block width
NQB = 4    # query blocks
NTT = 16   # 128-token tiles

f32 = mybir.dt.float32
bf16 = mybir.dt.bfloat16
BF = ml_dtypes.bfloat16


# ---------------------------------------------------------------- host prep

def _perm():
    """Column permutation: within each head's 64 dims, evens then odds."""
    p = np.empty(D, np.int64)
    for h in range(H):
        for m in range(32):
            p[h * 64 + m] = h * 64 + 2 * m
            p[h * 64 + 32 + m] = h * 64 + 2 * m + 1
    return p


def _rope_tables():
    """cos/sin tables [128, T] for the permuted (evens-first) layout."""
    inv = THETA ** (-(np.arange(0, DH, 2, dtype=np.float64) / DH))  # [32]
    m = np.arange(P) % 64
    fi = m % 32
    pos = np.arange(T)
    ang = pos[None, :].astype(np.float64) * inv[fi][:, None]  # [128, T]
    cos = np.cos(ang)
    sin = np.sin(ang) * np.where(m < 64 // 2, -1.0, 1.0)[:, None]
    return cos.astype(np.float32), sin.astype(np.float32)


def host_prep(x, Wq, bq, Wk, bk, Wv, bv, Wo, bo):
    """Build the 8 per-core input dicts (numpy, bf16)."""
    perm = _perm()
    WqTp = np.ascontiguousarray(Wq.T[:, perm]).astype(BF)
    WkTp = np.ascontiguousarray(Wk.T[:, perm]).astype(BF)
    WvT = np.ascontiguousarray(Wv.T).astype(BF)
    WoT = np.ascontiguousarray(Wo.T).astype(BF)
    bqp = bq[perm].astype(np.float32)
    bkp = bk[perm].astype(np.float32)
    ck, sk = _rope_tables()
    # the staircase mask strip is the same lower triangle for every tile
    r = np.arange(P)
    mk = (r[:, None] <= r[None, :]).astype(np.float32)
    # sel[r, oc, c]: broadcast-selector for the per-(oc) 1/den matmul:
    # row 2oc -> cols 0:64 (head h0), row 2oc+1 -> cols 64:128 (head h1)
    sel = np.zeros((8, OC, P), np.float32)
    for oc in range(OC):
        sel[2 * oc, oc, 0:64] = 1.0
        sel[2 * oc + 1, oc, 64:128] = 1.0
    sel = sel.reshape(8, OC * P)
    in_maps = []
    for c in range(NCORES):
        b, g = c // 2, c % 2
        gs = slice(DG * g, DG * (g + 1))
        in_maps.append({
            "xT": np.ascontiguousarray(x[b].T).astype(BF),
            "WqT": WqTp[:, gs], "WkT": WkTp[:, gs],
            "WvT": np.ascontiguousarray(WvT[:, gs]),
            "WoT": np.ascontiguousarray(WoT[gs, :]),
            "bq": bqp[gs].reshape(1, DG).astype(BF),
            "bk": bkp[gs].reshape(1, DG).astype(BF),
            "bv": bv[gs].reshape(1, DG).astype(BF),
            # host sums the two partials, so each adds half of bo
            "bo": (0.5 * bo).reshape(1, D).astype(BF),
            "ck": ck.astype(BF), "sk": sk.astype(BF),
            "mk": mk.astype(BF), "sel": sel.astype(BF),
        })
    return in_maps


def assemble(results):
    y = np.empty((B, T, D), np.float32)
    for b in range(B):
        y[b] = results[2 * b]["out"] + results[2 * b + 1]["out"]
    return y


# ------------------------------------------------------------- device build

def _legalize_waits(nc, max_waits=1):
    """Limit every instruction to one sync-wait command.

    Walrus's per-instruction structs encode a single sync wait; Tile can
    emit more. For any instruction with k > 1 waits, insert k-1 nops on
    the same engine immediately before it, each carrying one wait —
    position-preserving, so semantics are unchanged.
    """
    eng_obj = {
        mybir.EngineType.PE: nc.tensor,
        mybir.EngineType.Activation: nc.scalar,
        mybir.EngineType.DVE: nc.vector,
        mybir.EngineType.Pool: nc.gpsimd,
        mybir.EngineType.SP: nc.sync,
    }
    fn = nc.m.functions[0]
    for blk in fn.blocks:
        insts = list(blk.instructions)
        new = []
        for inst in insts:
            si = inst.sync_info
            nw = len(si.on_wait) if si is not None else 0
            if nw > max_waits:
                for w in si.on_wait[: nw - max_waits]:
                    eng_obj[inst.engine].nop()
                    nop = fn.blocks[-1].instructions[-1]
                    fn.blocks[-1].instructions = \
                        fn.blocks[-1].instructions[:-1]
                    nop.sync_info = mybir.SyncInfo(on_wait=[w], on_update=[])
                    new.append(nop)
                inst.sync_info = mybir.SyncInfo(
                    on_wait=list(si.on_wait[nw - max_waits:]),
                    on_update=list(si.on_update))
            new.append(inst)
        blk.instructions = new


def build_nc(use_bias):
    from contextlib import ExitStack

    nc = bass.Bass("TRN2", target_bir_lowering=False, debug=False,
                   num_devices=NCORES)
    Exp = mybir.ActivationFunctionType.Exp
    Ln = mybir.ActivationFunctionType.Ln

    xT = nc.dram_tensor("xT", [D, T], bf16, kind="ExternalInput").ap()
    WqT = nc.dram_tensor("WqT", [D, DG], bf16, kind="ExternalInput").ap()
    WkT = nc.dram_tensor("WkT", [D, DG], bf16, kind="ExternalInput").ap()
    WvT = nc.dram_tensor("WvT", [D, DG], bf16, kind="ExternalInput").ap()
    WoT = nc.dram_tensor("WoT", [DG, D], bf16, kind="ExternalInput").ap()
    if use_bias:
        bq_d = nc.dram_tensor("bq", [1, DG], bf16, kind="ExternalInput").ap()
        bk_d = nc.dram_tensor("bk", [1, DG], bf16, kind="ExternalInput").ap()
        bv_d = nc.dram_tensor("bv", [1, DG], bf16, kind="ExternalInput").ap()
        bo_d = nc.dram_tensor("bo", [1, D], bf16, kind="ExternalInput").ap()
    ck_d = nc.dram_tensor("ck", [P, T], bf16, kind="ExternalInput").ap()
    sk_d = nc.dram_tensor("sk", [P, T], bf16, kind="ExternalInput").ap()
    mk_d = nc.dram_tensor("mk", [P, P], bf16, kind="ExternalInput").ap()
    sel_d = nc.dram_tensor("sel", [8, OC * P], bf16,
                           kind="ExternalInput").ap()
    out_d = nc.dram_tensor("out", [T, D], f32, kind="ExternalOutput").ap()

    with tile.TileContext(nc) as tc, ExitStack() as ctx:
        big = ctx.enter_context(tc.tile_pool(name="big", bufs=1))
        const = ctx.enter_context(tc.tile_pool(name="const", bufs=1))
        rpool = ctx.enter_context(tc.tile_pool(name="rp", bufs=2))
        ppool = ctx.enter_context(tc.tile_pool(name="pp", bufs=4))
        npool = ctx.enter_context(tc.tile_pool(name="np", bufs=2))
        outp = ctx.enter_context(tc.tile_pool(name="outp", bufs=3))
        pssc = ctx.enter_context(
            tc.tile_pool(name="pssc", bufs=2, space="PSUM"))
        psacc = ctx.enter_context(
            tc.tile_pool(name="psacc", bufs=1, space="PSUM"))
        psmm = ctx.enter_context(
            tc.tile_pool(name="psmm", bufs=2, space="PSUM"))

        # ---- constants (scalar queue is idle early; demand order)
        ck_s = const.tile([P, T], bf16, tag="ck")
        nc.scalar.dma_start(ck_s, ck_d)
        sk_s = const.tile([P, T], bf16, tag="sk")
        nc.scalar.dma_start(sk_s, sk_d)
        mk_s = const.tile([P, P], bf16, tag="mk")
        nc.scalar.dma_start(mk_s, mk_d)
        if use_bias:
            bq_s = const.tile([1, DG], bf16, tag="bq")
            nc.scalar.dma_start(bq_s, bq_d)
            bk_s = const.tile([1, DG], bf16, tag="bk")
            nc.scalar.dma_start(bk_s, bk_d)
            bv_s = const.tile([1, DG], bf16, tag="bv")
            nc.scalar.dma_start(bv_s, bv_d)
            bo_s = const.tile([1, D], bf16, tag="bo")
            nc.scalar.dma_start(bo_s, bo_d)
            ones512 = const.tile([1, BLK], bf16, tag="ones512")
            nc.vector.memset(ones512, 1.0)
            onesb = const.tile([1, P], bf16, tag="onesb")
            nc.vector.memset(onesb, 1.0)

        # ---- weights / x (demand order: wk+x(tc0) -> wv -> wq -> ...)
        def load_w(src, n_in, n_col, tagp, q):
            tiles = []
            for dc in range(n_in // P):
                t = big.tile([P, n_col], bf16, tag=f"{tagp}{dc}")
                q(t, src[dc * P:(dc + 1) * P, :])
                tiles.append(t)
            return tiles
        # critical first bytes (wk + x tc0) split across two queues so the
        # first K-proj matmul can start as early as possible
        wk_s = []
        for dc in range(DC):
            wt = big.tile([P, DG], bf16, tag=f"wk{dc}")
            q = nc.gpsimd.dma_start if dc % 2 else nc.sync.dma_start
            q(wt, WkT[dc * P:(dc + 1) * P, :])
            wk_s.append(wt)
        x_s = []
        for dc in range(DC):
            xt = big.tile([P, T], bf16, tag=f"x{dc}")
            x_s.append(xt)
        for tcb in range(NQB):
            for dc in range(DC):
                q = nc.gpsimd.dma_start if dc % 2 else nc.sync.dma_start
                q(x_s[dc][:, tcb * BLK:(tcb + 1) * BLK],
                  xT[dc * P:(dc + 1) * P, tcb * BLK:(tcb + 1) * BLK])
        wv_s = load_w(WvT, D, DG, "wv", nc.scalar.dma_start)
        wq_s = load_w(WqT, D, DG, "wq", nc.scalar.dma_start)
        sel_s = const.tile([8, OC, P], bf16, tag="sel")
        nc.scalar.dma_start(sel_s, sel_d.rearrange("r (oc p) -> r oc p", p=P))
        wo_s = load_w(WoT, DG, D, "wo", nc.scalar.dma_start)

        qfin = big.tile([P, OC, T], bf16, tag="qfin")
        kfin = big.tile([P, OC, T], bf16, tag="kfin")
        vaug = big.tile([P, NTT, 8, 66], bf16, tag="vaug")
        nc.vector.memset(vaug[:, :, :, 0:1], 1.0)
        nc.vector.memset(vaug[:, :, :, 65:66], 1.0)
        ctxn = big.tile([P, OC, T], bf16, tag="ctxn")

        def rope(fin, oc, t_lo, wid=BLK):
            # rotate fin[:, oc, t_lo:t_lo+wid] in place (one producer)
            sl = slice(t_lo, t_lo + wid)
            sw = rpool.tile([P, T], bf16, tag="sw")
            for (a, src) in ((0, 32), (32, 0), (64, 96), (96, 64)):
                nc.gpsimd.dma_start(sw[a:a + 32, :wid],
                                    fin[src:src + 32, oc, sl])
            t1 = rpool.tile([P, T], bf16, tag="t1")
            t2 = rpool.tile([P, T], bf16, tag="t2")
            nc.vector.tensor_mul(t1[:, :wid], fin[:, oc, sl], ck_s[:, sl])
            nc.vector.tensor_mul(t2[:, :wid], sw[:, :wid], sk_s[:, sl])
            nc.vector.tensor_add(fin[:, oc, sl], t1[:, :wid], t2[:, :wid])

        def proj_qk(fin, w_tiles, b_s, oc, tcb, rope_now=True):
            # fin[:, oc, tcb*BLK:...] = (W^T x)[dims 128oc.., tokens] + rope
            ps = psmm.tile([P, BLK], f32, tag="mm")
            osl = slice(oc * P, (oc + 1) * P)
            tsl = slice(tcb * BLK, (tcb + 1) * BLK)
            for dc in range(DC):
                nc.tensor.matmul(ps, w_tiles[dc][:, osl], x_s[dc][:, tsl],
                                 start=(dc == 0),
                                 stop=(dc == DC - 1 and not use_bias))
            if use_bias:
                nc.tensor.matmul(ps, b_s[:, osl], ones512,
                                 start=False, stop=True)
            nc.vector.tensor_copy(fin[:, oc, tsl], ps)
            if rope_now:
                rope(fin, oc, tcb * BLK)

        def vproj(tt):
            ps = psmm.tile([P, DG], f32, tag="mm")
            for dc in range(DC):
                nc.tensor.matmul(ps, x_s[dc][:, tt * P:(tt + 1) * P],
                                 wv_s[dc],
                                 start=(dc == 0),
                                 stop=(dc == DC - 1 and not use_bias))
            if use_bias:
                nc.tensor.matmul(ps, onesb, bv_s, start=False, stop=True)
            nc.vector.tensor_copy(vaug[:, tt, 0:8, 1:65], ps)

        def oproj(qb, tp, half):
            tsl = slice(qb * BLK + tp * P, qb * BLK + (tp + 1) * P)
            esl = slice(half * BLK, (half + 1) * BLK)
            ps = psmm.tile([P, BLK], f32, tag="mm")
            for oc in range(OC):
                nc.tensor.matmul(ps, ctxn[:, oc, tsl], wo_s[oc][:, esl],
                                 start=(oc == 0),
                                 stop=(oc == OC - 1 and not use_bias))
            if use_bias:
                nc.tensor.matmul(ps, onesb, bo_s[:, esl],
                                 start=False, stop=True)
            ot = outp.tile([P, BLK], f32, tag="ot")
            nc.vector.tensor_copy(ot, ps)
            nc.sync.dma_start(out_d[tsl, esl], ot)

        def strip(ap2d, co):
            # cols [co:co+128] and [BLK+co:BLK+co+128] of a [128, 2*BLK] AP
            s = ap2d[:, co:]
            return bass.AP(tensor=s.tensor, offset=s.offset,
                           ap=[s.ap[0], [BLK, 2], [1, P]])

        def tri_mask():
            # the [128,128] triangle, read twice via a 0-stride middle dim
            s = mk_s[:, :]
            return bass.AP(tensor=s.tensor, offset=s.offset,
                           ap=[s.ap[0], [0, 2], [1, P]])

        # ---- stage: K proj (token-block outer, demand-ordered vs the x
        # DMA stream), batched full-T RoPE, V proj, Q proj(qb0)
        for tcb in range(NQB):
            for oc in range(OC):
                proj_qk(kfin, wk_s, bk_s if use_bias else None, oc, tcb,
                        rope_now=False)
        for oc in range(OC):
            rope(kfin, oc, 0, T)
        for tt in range(8):
            vproj(tt)
        for oc in range(OC):
            proj_qk(qfin, wq_s, bq_s if use_bias else None, oc, 0)

        # drip: PE work emitted between attention tiles (scalar paces
        # there). V for tokens [1024,2048) isn't needed until qb2.
        drip = [(lambda tt=tt: vproj(tt)) for tt in range(8, NTT)]

        # ---- main pipeline over query blocks
        for qb in range(NQB):
            qsl = slice(qb * BLK, (qb + 1) * BLK)
            J = 4 * qb + 4
            ntiles = OC * J
            period = max(1, ntiles // (len(drip) + 1))
            tile_i = 0
            stg = npool.tile([8, BLK], f32, tag="stg")
            for oc in range(OC):
                opsA = psacc.tile([P, BLK], f32, tag="opsA")
                opsB = psacc.tile([P, BLK], f32, tag="opsB")
                for jt in range(J):
                    # diagonal tiles: queries [0, co) of this block can't
                    # see kv tile jt — shrink every op to cols [co, BLK)
                    jl = jt - 4 * qb
                    co = jl * P if jl > 0 else 0
                    qco = slice(qb * BLK + co, (qb + 1) * BLK)
                    sAB = pssc.tile([P, 2 * BLK], f32, tag="sAB")
                    nc.tensor.matmul(
                        sAB[:, co:BLK],
                        kfin[0:64, oc, jt * P:(jt + 1) * P],
                        qfin[0:64, oc, qco],
                        start=True, stop=True, tile_position=(0, 0))
                    nc.tensor.matmul(
                        sAB[:, BLK + co:2 * BLK],
                        kfin[64:128, oc, jt * P:(jt + 1) * P],
                        qfin[64:128, oc, qco],
                        start=True, stop=True, tile_position=(64, 0))
                    pAB = ppool.tile([P, 2 * BLK], bf16, tag="pAB")
                    if co > 0:
                        s_in = sAB[:, co:]
                        s_src = bass.AP(
                            tensor=s_in.tensor, offset=s_in.offset,
                            ap=[s_in.ap[0], [BLK, 2], [1, BLK - co]])
                        p_out = pAB[:, co:]
                        p_dst = bass.AP(
                            tensor=p_out.tensor, offset=p_out.offset,
                            ap=[p_out.ap[0], [BLK, 2], [1, BLK - co]])
                        nc.scalar.activation(p_dst, s_src, Exp, scale=0.125)
                    else:
                        nc.scalar.activation(pAB, sAB, Exp, scale=0.125)
                    if jl >= 0:
                        # staircase mask only on the diagonal strip
                        nc.vector.tensor_mul(strip(pAB, co), strip(pAB, co),
                                             tri_mask())
                    nc.tensor.matmul(opsA[0:65, co:BLK],
                                     vaug[:, jt, 2 * oc, 1:66],
                                     pAB[:, co:BLK],
                                     start=(jt == 0), stop=(jt == J - 1))
                    nc.tensor.matmul(opsB[0:65, co:BLK],
                                     vaug[:, jt, 2 * oc + 1, 1:66],
                                     pAB[:, BLK + co:2 * BLK],
                                     start=(jt == 0), stop=(jt == J - 1))
                    tile_i += 1
                    if len(drip) > 2 and tile_i % period == 0:
                        drip.pop(0)()
                # evacuate unnormalized ctx (releases opsA/B for next oc)
                # and stage the two denominator rows into rows 2oc,2oc+1
                nc.vector.tensor_copy(ctxn[0:64, oc, qsl], opsA[0:64, :])
                nc.vector.tensor_copy(ctxn[64:128, oc, qsl], opsB[0:64, :])
                for (hh, ops) in ((0, opsA), (1, opsB)):
                    dt = npool.tile([1, BLK], f32, tag="dtmp")
                    nc.vector.tensor_copy(dt, ops[64:65, :])
                    nc.sync.dma_start(stg[2 * oc + hh:2 * oc + hh + 1, :], dt)
            for g in drip:
                g()
            drip = []
            # Q proj for the next block covers the normalize chain
            if qb < NQB - 1:
                for oc in range(OC):
                    proj_qk(qfin, wq_s, bq_s if use_bias else None,
                            oc, qb + 1)
            # batched reciprocal: rec = exp(-ln(den)), then per-oc broadcast
            lnd = npool.tile([8, BLK], f32, tag="lnd")
            nc.scalar.activation(lnd, stg, Ln)
            rec = npool.tile([8, BLK], bf16, tag="rec")
            nc.scalar.activation(rec, lnd, Exp, scale=-1.0)
            for oc in range(OC):
                bc = psmm.tile([P, BLK], f32, tag="mm")
                nc.tensor.matmul(bc, sel_s[:, oc, :], rec,
                                 start=True, stop=True)
                nc.vector.tensor_mul(ctxn[0:64, oc, qsl],
                                     ctxn[0:64, oc, qsl], bc[0:64, :])
                nc.vector.tensor_mul(ctxn[64:128, oc, qsl],
                                     ctxn[64:128, oc, qsl], bc[64:128, :])
            # out-proj groups drip into the next qb's attention
            for tp in range(4):
                for half in range(2):
                    drip.append(lambda q=qb, tp=tp, h=half: oproj(q, tp, h))
        for g in drip:
            g()
    _legalize_waits(nc)
    return nc


# ------------------------------------------------------------------- entry

def kernel(x, Wq, bq, Wk, bk, Wv, bv, Wo, bo):
    x = np.asarray(x, np.float32)
    Wq, bq = np.asarray(Wq, np.float32), np.asarray(bq, np.float32)
    Wk, bk = np.asarray(Wk, np.float32), np.asarray(bk, np.float32)
    Wv, bv = np.asarray(Wv, np.float32), np.asarray(bv, np.float32)
    Wo, bo = np.asarray(Wo, np.float32), np.asarray(bo, np.float32)
    use_bias = bool(any(np.any(b) for b in (bq, bk, bv, bo)))
    in_maps = host_prep(x, Wq, bq, Wk, bk, Wv, bv, Wo, bo)
    if not use_bias:
        for m in in_maps:
            for k in ("bq", "bk", "bv", "bo"):
                m.pop(k)
    nc = build_nc(use_bias)
    res = run_bass_kernel_spmd(nc, in_maps, list(range(NCORES))).results
    return assemble(res)
